# revision 15
# baseline (speedup 1.0000x reference)
"""MinGRU (2-layer) Trainium2 Bass kernel.

Problem: B=8, S=4096, D=H=1024.
  layer(inp, W, b): gh = inp @ W.T + b ; gate, hid = split(gh)
    z = sigmoid(gate); a = 1 - z = sigmoid(-gate)
    g = where(hid >= 0, hid + 0.5, sigmoid(hid)) = relu(hid) + min(sigmoid(hid), 0.5)
    h_t = a_t * h_{t-1} + z_t * g_t        (h_0 = 0.5)
  out = layer(layer(x, W0, b0), W1, b1)

Sharding: data-parallel over batch, one batch per NeuronCore (8 cores).

Per-core dataflow (batch b):
  - host pre-transposes x[b] -> xT (D, S) so the contraction dim lies on
    SBUF partitions; weights pre-transposed to W^T (D, 2H) on host too.
  - matmuls run in float32r (fp32 data, replicated-read PE mode: ~1 cy/row
    at N=512, max rel err ~1.6e-4) producing gh^T tiles (features x seq).
  - gate/hidden nonlinearities on ScalarE (sigmoid/relu with fused
    per-partition bias), z/g/b on VectorE (g,b in-place to save SBUF).
  - the recurrence h_t = a_t h_{t-1} + b_t is one VectorE
    tensor_tensor_scan per (128-feature, 512-seq) tile, chained across
    seq chunks via initial=prev[:, -1:]; bit-exact vs a sequential fp32 scan.
  - layer-0 output h1^T streams through a DRAM scratch; layer 1 reads it
    back as its rhs. Both weight sets stay resident in SBUF; W1^T k-tiles
    are prefetched staggered between layer-0 chunks so the layer
    transition has no DMA bubble.
  - read DMAs ride the SP HWDGE ring, write DMAs the ACT ring, so
    h1^T/out writebacks never queue ahead of the next chunk's reads.
  - layer-1 output is transposed back to (seq, features) via PE-transpose
    into PSUM, copied to SBUF, and DMA'd to the (S, H) output.
"""
import sys

sys.path.insert(0, "/opt/trn_rl_repo")

import numpy as np
from contextlib import ExitStack

from concourse import bacc, tile, mybir

dt = mybir.dt
Alu = mybir.AluOpType
Act = mybir.ActivationFunctionType

B, S, D, H = 8, 4096, 1024, 1024
SC = 512                # seq chunk (PSUM-bank-sized matmul N)
NCH = S // SC           # 8 chunks
NKT = D // 128          # 8 contraction tiles
NFB = H // 128          # 8 feature blocks (gate rows i, hidden rows i+8)

_cached = {}


def _build():
    nc = bacc.Bacc("TRN2", target_bir_lowering=False, debug=False, num_devices=8)

    d_xT = nc.dram_tensor("xT", [D, S], dt.float32r, kind="ExternalInput").ap()
    d_w0 = nc.dram_tensor("w0T", [D, 2 * H], dt.float32r, kind="ExternalInput").ap()
    d_w1 = nc.dram_tensor("w1T", [D, 2 * H], dt.float32r, kind="ExternalInput").ap()
    # bias columns: (128, NFB); col i = bias slice for feature block i
    d_bg0 = nc.dram_tensor("bg0n", [128, NFB], dt.float32, kind="ExternalInput").ap()
    d_bh0 = nc.dram_tensor("bh0", [128, NFB], dt.float32, kind="ExternalInput").ap()
    d_bg1 = nc.dram_tensor("bg1n", [128, NFB], dt.float32, kind="ExternalInput").ap()
    d_bh1 = nc.dram_tensor("bh1", [128, NFB], dt.float32, kind="ExternalInput").ap()
    d_id = nc.dram_tensor("ident", [128, 128], dt.float32, kind="ExternalInput").ap()
    d_out = nc.dram_tensor("out", [S, H], dt.float32, kind="ExternalOutput").ap()

    with tile.TileContext(nc) as tc, ExitStack() as ctx:
        cpool = ctx.enter_context(tc.tile_pool(name="const", bufs=1))
        dpool = ctx.enter_context(tc.tile_pool(name="dram", bufs=1, space="DRAM"))
        wpool = ctx.enter_context(tc.tile_pool(name="w", bufs=1))
        rpool = ctx.enter_context(tc.tile_pool(name="rhs", bufs=2))
        tpool = ctx.enter_context(tc.tile_pool(name="tmp", bufs=2))
        hpool = ctx.enter_context(tc.tile_pool(name="h", bufs=2))
        opool = ctx.enter_context(tc.tile_pool(name="o", bufs=1))
        pspool = ctx.enter_context(tc.tile_pool(name="ps", bufs=2, space="PSUM"))

        t_bg0 = cpool.tile([128, NFB], dt.float32)
        nc.scalar.dma_start(t_bg0[:], d_bg0)
        t_bh0 = cpool.tile([128, NFB], dt.float32)
        nc.scalar.dma_start(t_bh0[:], d_bh0)
        t_bg1 = cpool.tile([128, NFB], dt.float32)
        nc.scalar.dma_start(t_bg1[:], d_bg1)
        t_bh1 = cpool.tile([128, NFB], dt.float32)
        nc.scalar.dma_start(t_bh1[:], d_bh1)
        t_id = cpool.tile([128, 128], dt.float32)
        nc.scalar.dma_start(t_id[:], d_id)

        h1T = dpool.tile([D, S], dt.float32r, name="h1T")

        # Both layers' weights stay resident: [128, NKT * 2H] each,
        # k-tiles along the free axis.
        t_w0 = wpool.tile([128, NKT * 2 * H], dt.float32r, name="w0t")
        t_w1 = wpool.tile([128, NKT * 2 * H], dt.float32r, name="w1t")

        def load_w(t_w, d_w, k):
            nc.sync.dma_start(
                t_w[:, 2 * H * k : 2 * H * (k + 1)], d_w[128 * k : 128 * (k + 1), :]
            )

        load_w(t_w0, d_w0, 0)  # first k-tile first so PE can start early

        w0_v = t_w0[:].rearrange("p (k e) -> k p e", k=NKT)
        w1_v = t_w1[:].rearrange("p (k e) -> k p e", k=NKT)

        def do_layer(src_dram, w_v, t_bg, t_bh, is_last, prefetch=None):
            carry = [None] * NFB
            for c in range(NCH):
                rhs = []
                for k in range(NKT):
                    t = rpool.tile(
                        [128, SC],
                        dt.float32r,
                        name=f"rhs{k}",
                        tag=f"rhs{k}",
                        bufs=2,
                    )
                    nc.sync.dma_start(
                        t[:], src_dram[128 * k : 128 * (k + 1), SC * c : SC * (c + 1)]
                    )
                    rhs.append(t)
                if prefetch is not None:
                    prefetch(c)

                htiles = []
                for i in range(NFB):
                    ps_g = pspool.tile([128, SC], dt.float32, name="psg", tag="psg", bufs=3)
                    for k in range(NKT):
                        nc.tensor.matmul(
                            ps_g[:],
                            w_v[k, :, 128 * i : 128 * (i + 1)],
                            rhs[k][:],
                            start=(k == 0),
                            stop=(k == NKT - 1),
                        )
                    ps_h = pspool.tile([128, SC], dt.float32, name="psh", tag="psh", bufs=3)
                    for k in range(NKT):
                        nc.tensor.matmul(
                            ps_h[:],
                            w_v[k, :, 128 * (i + NFB) : 128 * (i + NFB + 1)],
                            rhs[k][:],
                            start=(k == 0),
                            stop=(k == NKT - 1),
                        )
                    # a = sigmoid(-(gate + b_g)) ; bias col already negated
                    a = tpool.tile([128, SC], dt.float32, name="a", tag="a")
                    nc.scalar.activation(
                        a[:], ps_g[:], Act.Sigmoid, bias=t_bg[:, i : i + 1], scale=-1.0
                    )
                    s1 = tpool.tile([128, SC], dt.float32, name="s1", tag="s1", bufs=1)
                    nc.scalar.activation(
                        s1[:], ps_h[:], Act.Sigmoid, bias=t_bh[:, i : i + 1]
                    )
                    r = tpool.tile([128, SC], dt.float32, name="r", tag="r")
                    nc.scalar.activation(
                        r[:], ps_h[:], Act.Relu, bias=t_bh[:, i : i + 1]
                    )
                    # g = min(s1, 0.5) + r   (in place onto r)
                    nc.vector.scalar_tensor_tensor(
                        r[:], s1[:], 0.5, r[:], op0=Alu.min, op1=Alu.add
                    )
                    # bneg = (a - 1) * g = -z*g   (in place onto r)
                    nc.vector.scalar_tensor_tensor(
                        r[:], a[:], 1.0, r[:], op0=Alu.subtract, op1=Alu.mult
                    )
                    # recurrence: h = a * h_prev - bneg = a*h_prev + z*g
                    ho = hpool.tile(
                        [128, SC],
                        dt.float32 if is_last else dt.float32r,
                        name=f"h{i}",
                        tag=f"h{i}",
                    )
                    init = 0.5 if c == 0 else carry[i][:, SC - 1 : SC]
                    nc.vector.tensor_tensor_scan(
                        ho[:], a[:], r[:], init, op0=Alu.mult, op1=Alu.subtract
                    )
                    carry[i] = ho
                    htiles.append(ho)
                    if not is_last:
                        nc.scalar.dma_start(
                            h1T[128 * i : 128 * (i + 1), SC * c : SC * (c + 1)],
                            ho[:],
                        )
                if is_last:
                    # transpose h-chunk (H x SC) -> (SC x H) and write out
                    for j in range(SC // 128):
                        ps_o = pspool.tile([128, H], dt.float32, name="pso", tag="pso", bufs=1)
                        for i in range(NFB):
                            nc.tensor.transpose(
                                ps_o[:, 128 * i : 128 * (i + 1)],
                                htiles[i][:, 128 * j : 128 * (j + 1)],
                                t_id[:],
                            )
                        s0 = SC * c + 128 * j
                        for half in range(2):
                            so = opool.tile(
                                [128, H // 2], dt.float32, name="so", tag=f"so{half}"
                            )
                            nc.scalar.copy(
                                so[:], ps_o[:, (H // 2) * half : (H // 2) * (half + 1)]
                            )
                            nc.scalar.dma_start(
                                d_out[
                                    s0 : s0 + 128,
                                    (H // 2) * half : (H // 2) * (half + 1),
                                ],
                                so[:],
                            )

        def prefetch_l0(c):
            # rest of W0 right after chunk 0's rhs, then W1 staggered over
            # the remaining NCH-1 chunks (two k-tiles on the first one)
            if c == 0:
                for k in range(1, NKT):
                    load_w(t_w0, d_w0, k)
            elif c == 1:
                load_w(t_w1, d_w1, 0)
                load_w(t_w1, d_w1, 1)
            elif c < NCH:
                load_w(t_w1, d_w1, c)

        do_layer(d_xT, w0_v, t_bg0, t_bh0, is_last=False, prefetch=prefetch_l0)
        do_layer(h1T[:], w1_v, t_bg1, t_bh1, is_last=True)

    nc.compile()
    return nc


def _prep_shared(W0, b0, W1, b1):
    def bias_cols(bvec):
        return np.ascontiguousarray(bvec.reshape(NFB, 128).T.astype(np.float32))

    return {
        "w0T": np.ascontiguousarray(W0.T.astype(np.float32)),
        "w1T": np.ascontiguousarray(W1.T.astype(np.float32)),
        "bg0n": bias_cols(-b0[:H]),
        "bh0": bias_cols(b0[H:]),
        "bg1n": bias_cols(-b1[:H]),
        "bh1": bias_cols(b1[H:]),
        "ident": np.eye(128, dtype=np.float32),
    }


def kernel(x, W0, b0, W1, b1):
    from concourse.bass_utils import run_bass_kernel_spmd

    if "nc" not in _cached:
        _cached["nc"] = _build()
    nc = _cached["nc"]

    x = np.asarray(x)
    W0, b0, W1, b1 = (np.asarray(t) for t in (W0, b0, W1, b1))
    shared = _prep_shared(W0, b0, W1, b1)
    in_maps = []
    for b in range(B):
        m = dict(shared)
        m["xT"] = np.ascontiguousarray(np.asarray(x)[b].T.astype(np.float32))
        in_maps.append(m)

    res = run_bass_kernel_spmd(nc, in_maps, core_ids=list(range(B)))
    out = np.stack([res.results[b]["out"] for b in range(B)], axis=0)
    return out


# revision 16
# speedup vs baseline: 1.1665x; 1.1665x over previous
"""MinGRU (2-layer) Trainium2 Bass kernel.

Problem: B=8, S=4096, D=H=1024.
  layer(inp, W, b): gh = inp @ W.T + b ; gate, hid = split(gh)
    z = sigmoid(gate); a = 1 - z = sigmoid(-gate)
    g = where(hid >= 0, hid + 0.5, sigmoid(hid)) = relu(hid) + min(sigmoid(hid), 0.5)
    h_t = a_t * h_{t-1} + z_t * g_t        (h_0 = 0.5)
  out = layer(layer(x, W0, b0), W1, b1)

Sharding: data-parallel over batch, one batch per NeuronCore (8 cores).

Per-core dataflow (batch b):
  - host pre-transposes x[b] -> xT (D, S) so the contraction dim lies on
    SBUF partitions; weights pre-transposed to W^T (D, 2H) on host too.
  - matmuls run in float32r (fp32 data, replicated-read PE mode: ~1 cy/row
    at N=512, max rel err ~1.6e-4) producing gh^T tiles (features x seq).
  - gate/hidden nonlinearities on ScalarE (sigmoid/relu with fused
    per-partition bias), z/g/b on VectorE (g,b in-place to save SBUF).
  - the recurrence h_t = a_t h_{t-1} + b_t is one VectorE
    tensor_tensor_scan per (128-feature, 512-seq) tile, chained across
    seq chunks via initial=prev[:, -1:]; bit-exact vs a sequential fp32 scan.
  - layer-0 output h1^T streams through a DRAM scratch; layer 1 reads it
    back as its rhs. Both weight sets stay resident in SBUF; W1^T k-tiles
    are prefetched staggered between layer-0 chunks so the layer
    transition has no DMA bubble.
  - read DMAs ride the SP HWDGE ring, write DMAs the ACT ring, so
    h1^T/out writebacks never queue ahead of the next chunk's reads.
  - layer-1 output is transposed back to (seq, features) via PE-transpose
    into PSUM, copied to SBUF, and DMA'd to the (S, H) output.
"""
import sys

sys.path.insert(0, "/opt/trn_rl_repo")

import numpy as np
from contextlib import ExitStack

from concourse import bacc, tile, mybir

dt = mybir.dt
Alu = mybir.AluOpType
Act = mybir.ActivationFunctionType

B, S, D, H = 8, 4096, 1024, 1024
SC = 512                # seq chunk (PSUM-bank-sized matmul N)
NCH = S // SC           # 8 chunks
NKT = D // 128          # 8 contraction tiles
NFB = H // 128          # 8 feature blocks (gate rows i, hidden rows i+8)

_cached = {}


def _build():
    nc = bacc.Bacc("TRN2", target_bir_lowering=False, debug=False, num_devices=8)

    d_xT = nc.dram_tensor("xT", [D, S], dt.float32r, kind="ExternalInput").ap()
    d_w0 = nc.dram_tensor("w0T", [D, 2 * H], dt.float32r, kind="ExternalInput").ap()
    d_w1 = nc.dram_tensor("w1T", [D, 2 * H], dt.float32r, kind="ExternalInput").ap()
    # bias columns: (128, NFB); col i = bias slice for feature block i
    d_bg0 = nc.dram_tensor("bg0n", [128, NFB], dt.float32, kind="ExternalInput").ap()
    d_bh0 = nc.dram_tensor("bh0", [128, NFB], dt.float32, kind="ExternalInput").ap()
    d_bg1 = nc.dram_tensor("bg1n", [128, NFB], dt.float32, kind="ExternalInput").ap()
    d_bh1 = nc.dram_tensor("bh1", [128, NFB], dt.float32, kind="ExternalInput").ap()
    d_id = nc.dram_tensor("ident", [128, 128], dt.float32, kind="ExternalInput").ap()
    d_out = nc.dram_tensor("out", [S, H], dt.float32, kind="ExternalOutput").ap()

    with tile.TileContext(nc) as tc, ExitStack() as ctx:
        cpool = ctx.enter_context(tc.tile_pool(name="const", bufs=1))
        dpool = ctx.enter_context(tc.tile_pool(name="dram", bufs=1, space="DRAM"))
        wpool = ctx.enter_context(tc.tile_pool(name="w", bufs=1))
        rpool = ctx.enter_context(tc.tile_pool(name="rhs", bufs=2))
        tpool = ctx.enter_context(tc.tile_pool(name="tmp", bufs=2))
        hpool = ctx.enter_context(tc.tile_pool(name="h", bufs=2))
        opool = ctx.enter_context(tc.tile_pool(name="o", bufs=1))
        pspool = ctx.enter_context(tc.tile_pool(name="ps", bufs=2, space="PSUM"))

        t_bg0 = cpool.tile([128, NFB], dt.float32)
        nc.scalar.dma_start(t_bg0[:], d_bg0)
        t_bh0 = cpool.tile([128, NFB], dt.float32)
        nc.scalar.dma_start(t_bh0[:], d_bh0)
        t_bg1 = cpool.tile([128, NFB], dt.float32)
        nc.scalar.dma_start(t_bg1[:], d_bg1)
        t_bh1 = cpool.tile([128, NFB], dt.float32)
        nc.scalar.dma_start(t_bh1[:], d_bh1)
        t_id = cpool.tile([128, 128], dt.float32)
        nc.scalar.dma_start(t_id[:], d_id)

        h1T = dpool.tile([D, S], dt.float32r, name="h1T")

        # Both layers' weights stay resident: [128, NKT * 2H] each,
        # k-tiles along the free axis.
        t_w0 = wpool.tile([128, NKT * 2 * H], dt.float32r, name="w0t")
        t_w1 = wpool.tile([128, NKT * 2 * H], dt.float32r, name="w1t")

        def load_w(t_w, d_w, k):
            nc.sync.dma_start(
                t_w[:, 2 * H * k : 2 * H * (k + 1)], d_w[128 * k : 128 * (k + 1), :]
            )

        load_w(t_w0, d_w0, 0)  # first k-tile first so PE can start early

        w0_v = t_w0[:].rearrange("p (k e) -> k p e", k=NKT)
        w1_v = t_w1[:].rearrange("p (k e) -> k p e", k=NKT)

        def do_layer(src_dram, w_v, t_bg, t_bh, is_last, prefetch=None):
            carry = [None] * NFB
            for c in range(NCH):
                rhs = []
                for k in range(NKT):
                    t = rpool.tile(
                        [128, SC],
                        dt.float32r,
                        name=f"rhs{k}",
                        tag=f"rhs{k}",
                        bufs=2,
                    )
                    nc.sync.dma_start(
                        t[:], src_dram[128 * k : 128 * (k + 1), SC * c : SC * (c + 1)]
                    )
                    rhs.append(t)
                if prefetch is not None:
                    prefetch(c)

                htiles = []
                for i in range(NFB):
                    ps_g = pspool.tile([128, SC], dt.float32, name="psg", tag="psg")
                    for k in range(NKT):
                        nc.tensor.matmul(
                            ps_g[:],
                            w_v[k, :, 128 * i : 128 * (i + 1)],
                            rhs[k][:],
                            start=(k == 0),
                            stop=(k == NKT - 1),
                        )
                    ps_h = pspool.tile([128, SC], dt.float32, name="psh", tag="psh")
                    for k in range(NKT):
                        nc.tensor.matmul(
                            ps_h[:],
                            w_v[k, :, 128 * (i + NFB) : 128 * (i + NFB + 1)],
                            rhs[k][:],
                            start=(k == 0),
                            stop=(k == NKT - 1),
                        )
                    # a = sigmoid(-(gate + b_g)) ; bias col already negated
                    a = tpool.tile([128, SC], dt.float32, name="a", tag="a")
                    nc.scalar.activation(
                        a[:], ps_g[:], Act.Sigmoid, bias=t_bg[:, i : i + 1], scale=-1.0
                    )
                    s1 = tpool.tile([128, SC], dt.float32, name="s1", tag="s1", bufs=1)
                    nc.scalar.activation(
                        s1[:], ps_h[:], Act.Sigmoid, bias=t_bh[:, i : i + 1]
                    )
                    r = tpool.tile([128, SC], dt.float32, name="r", tag="r")
                    nc.scalar.activation(
                        r[:], ps_h[:], Act.Relu, bias=t_bh[:, i : i + 1]
                    )
                    # g = min(s1, 0.5) + r   (in place onto r)
                    nc.vector.scalar_tensor_tensor(
                        r[:], s1[:], 0.5, r[:], op0=Alu.min, op1=Alu.add
                    )
                    # bneg = (a - 1) * g = -z*g   (in place onto r)
                    nc.vector.scalar_tensor_tensor(
                        r[:], a[:], 1.0, r[:], op0=Alu.subtract, op1=Alu.mult
                    )
                    # recurrence: h = a * h_prev - bneg = a*h_prev + z*g
                    ho = hpool.tile(
                        [128, SC],
                        dt.float32 if is_last else dt.float32r,
                        name=f"h{i}",
                        tag=f"h{i}",
                    )
                    init = 0.5 if c == 0 else carry[i][:, SC - 1 : SC]
                    nc.vector.tensor_tensor_scan(
                        ho[:], a[:], r[:], init, op0=Alu.mult, op1=Alu.subtract
                    )
                    carry[i] = ho
                    htiles.append(ho)
                    if not is_last:
                        nc.scalar.dma_start(
                            h1T[128 * i : 128 * (i + 1), SC * c : SC * (c + 1)],
                            ho[:],
                        )
                if is_last:
                    # transpose h-chunk (H x SC) -> (SC x H) and write out
                    for j in range(SC // 128):
                        ps_o = pspool.tile([128, H], dt.float32, name="pso", tag="pso")
                        for i in range(NFB):
                            nc.tensor.transpose(
                                ps_o[:, 128 * i : 128 * (i + 1)],
                                htiles[i][:, 128 * j : 128 * (j + 1)],
                                t_id[:],
                            )
                        s0 = SC * c + 128 * j
                        for half in range(2):
                            so = opool.tile(
                                [128, H // 2], dt.float32, name="so", tag=f"so{half}"
                            )
                            nc.scalar.copy(
                                so[:], ps_o[:, (H // 2) * half : (H // 2) * (half + 1)]
                            )
                            nc.scalar.dma_start(
                                d_out[
                                    s0 : s0 + 128,
                                    (H // 2) * half : (H // 2) * (half + 1),
                                ],
                                so[:],
                            )

        def prefetch_l0(c):
            # rest of W0 right after chunk 0's rhs, then W1 staggered over
            # the remaining NCH-1 chunks (two k-tiles on the first one)
            if c == 0:
                for k in range(1, NKT):
                    load_w(t_w0, d_w0, k)
            elif c == 1:
                load_w(t_w1, d_w1, 0)
                load_w(t_w1, d_w1, 1)
            elif c < NCH:
                load_w(t_w1, d_w1, c)

        do_layer(d_xT, w0_v, t_bg0, t_bh0, is_last=False, prefetch=prefetch_l0)
        do_layer(h1T[:], w1_v, t_bg1, t_bh1, is_last=True)

    nc.compile()
    return nc


def _prep_shared(W0, b0, W1, b1):
    def bias_cols(bvec):
        return np.ascontiguousarray(bvec.reshape(NFB, 128).T.astype(np.float32))

    return {
        "w0T": np.ascontiguousarray(W0.T.astype(np.float32)),
        "w1T": np.ascontiguousarray(W1.T.astype(np.float32)),
        "bg0n": bias_cols(-b0[:H]),
        "bh0": bias_cols(b0[H:]),
        "bg1n": bias_cols(-b1[:H]),
        "bh1": bias_cols(b1[H:]),
        "ident": np.eye(128, dtype=np.float32),
    }


def kernel(x, W0, b0, W1, b1):
    from concourse.bass_utils import run_bass_kernel_spmd

    if "nc" not in _cached:
        _cached["nc"] = _build()
    nc = _cached["nc"]

    x = np.asarray(x)
    W0, b0, W1, b1 = (np.asarray(t) for t in (W0, b0, W1, b1))
    shared = _prep_shared(W0, b0, W1, b1)
    in_maps = []
    for b in range(B):
        m = dict(shared)
        m["xT"] = np.ascontiguousarray(np.asarray(x)[b].T.astype(np.float32))
        in_maps.append(m)

    res = run_bass_kernel_spmd(nc, in_maps, core_ids=list(range(B)))
    out = np.stack([res.results[b]["out"] for b in range(B)], axis=0)
    return out
